# revision 15
# baseline (speedup 1.0000x reference)
"""CrossViewAttention Trainium2 kernel (bf16), v3.

Sharding: B*V=16 instances over 8 cores, 2 per core paired as (v, v+2) so
both share view v+1's K/V projection -> per-core KV tokens 768 instead of
1024 (views v-1, v+1, v+3), cutting KV-projection matmul rows 25%.

QK^T contracts over only HD=64 (half the 128x128 PE idle), so the two
kv-heads of a block are packed into partition halves of KT/QTP and issued
as two matmuls in separate PE row-groups (tile_position row tiling) that
stream concurrently. Their two PSUM banks are one [128,1024] tile, so a
single scalar-engine exp covers both (halving exp instruction overhead
and keeping the scalar engine off the critical path).

DMA uses both hardware queues: weights for phase A plus x_q go out on the
scalar engine's queue (idle during A), everything else on sync.

Pipeline per core:
  A1: KT[m][128,768] = wk.T @ x_kv^T   (kv-heads 2m / 2m+1 in halves)
  A2: VA[tb][128,8*65] = x_kv @ wv     (+ ones col for softmax denom)
  A3: QTP[m][128,2048] = wq.T @ x^T    (interleaved with attention)
  B:  per (m,p,chunk,tt): packed QK pair -> fused exp -> PV (lag-1)
  C:  y = O @ wo (first chunk interleaved into the last attention block)
"""
import numpy as np

B, V, S, D = 2, 8, 256, 2048
NH, NKV, KVR = 32, 8, 2
HD = D // NH  # 64
G = NH // NKV  # 4
N_CORES = 8
P = 2  # pairs per core
TKV = 768  # kv tokens per core (3 shared views)
SCALE = 1.0 / np.sqrt(HD)

# core -> (vA, vB) view pairs (vB = vA + 2, sharing view vA+1's KV)
_VIEW_PAIRS = [(0, 2), (1, 3), (4, 6), (5, 7)]

_CACHE = {}


def _build():
    import concourse.bass as bass
    import concourse.tile as tile
    import concourse.mybir as mybir
    from concourse import bacc
    from contextlib import ExitStack

    F32 = mybir.dt.float32
    BF16 = mybir.dt.bfloat16

    nc = bacc.Bacc("TRN2", target_bir_lowering=False, debug=False,
                   num_devices=N_CORES)
    xqT = nc.dram_tensor("xqT", [D, P * S], BF16, kind="ExternalInput").ap()
    xkvT = nc.dram_tensor("xkvT", [D, TKV], BF16, kind="ExternalInput").ap()
    wq = nc.dram_tensor("wq", [D, D], BF16, kind="ExternalInput").ap()
    wkv = nc.dram_tensor("wkv", [D, 1024], BF16, kind="ExternalInput").ap()
    wo = nc.dram_tensor("wo", [D, D], BF16, kind="ExternalInput").ap()
    y = nc.dram_tensor("y", [P * S, D], BF16, kind="ExternalOutput").ap()

    with tile.TileContext(nc) as tc, ExitStack() as top:
        ktp = top.enter_context(tc.tile_pool(name="ktp", bufs=1))
        vp = top.enter_context(tc.tile_pool(name="vp", bufs=1))

        # KT[m]: rows 0-63 kv-head 2m, rows 64-127 kv-head 2m+1; cols = t
        KT = [ktp.tile([128, TKV], BF16, tag=f"kt{m}", name=f"kt{m}")
              for m in range(4)]
        # VA[tb]: t-block tb (128 tokens) x (8 kv-heads x (64 hd + 1 one))
        VA = [vp.tile([128, 8 * 65], BF16, tag=f"va{tb}", name=f"va{tb}")
              for tb in range(6)]
        for tb in range(6):
            od = VA[tb][:].rearrange("q (h c) -> q h c", c=65)[:, :, 64:65]
            nc.gpsimd.memset(od, 1.0)

        # ---------- Phase A1/A2: K^T, V ----------
        with ExitStack() as ph:
            xkp = ph.enter_context(tc.tile_pool(name="xkp", bufs=1))
            wvp = ph.enter_context(tc.tile_pool(name="wvp", bufs=6))
            wst = ph.enter_context(tc.tile_pool(name="wst", bufs=6))
            psA = ph.enter_context(tc.tile_pool(name="psA", bufs=8, space="PSUM"))

            xkv = [xkp.tile([128, TKV], BF16, tag=f"xkv{k}", name=f"xkv{k}")
                   for k in range(16)]

            # A1: KT accumulation, 8 psum banks (4 m x 2 halves of 384)
            kps = [psA.tile([128, 512], F32, tag="pa", name=f"kps{i}")
                   for i in range(8)]
            for k in range(16):
                nc.sync.dma_start(xkv[k][:], xkvT[k * 128:(k + 1) * 128, :])
                wt = wst.tile([128, 512], BF16, tag="wk")
                nc.scalar.dma_start(wt[:], wkv[k * 128:(k + 1) * 128, 0:512])
                for m in range(4):
                    for h in range(2):
                        nc.tensor.matmul(
                            kps[m * 2 + h][:, 0:384],
                            wt[:, m * 128:(m + 1) * 128],
                            xkv[k][:, h * 384:(h + 1) * 384],
                            start=(k == 0), stop=(k == 15))
            for m in range(4):
                for h in range(2):
                    dst = KT[m][:, h * 384:(h + 1) * 384]
                    src = kps[m * 2 + h][:, 0:384]
                    if h == 0:
                        nc.vector.tensor_copy(dst, src)
                    else:
                        nc.scalar.copy(dst, src)

            # A2: V natural layout, 6 psum banks (t-blocks)
            vps = [psA.tile([128, 512], F32, tag="pa", name=f"pvv{tb}")
                   for tb in range(6)]
            for k in range(16):
                wvt = wvp.tile([128, 512], BF16, tag="wv")
                nc.scalar.dma_start(wvt[:], wkv[k * 128:(k + 1) * 128, 512:1024])
                for tb in range(6):
                    nc.tensor.matmul(
                        vps[tb][:],
                        xkv[k][:, tb * 128:(tb + 1) * 128],
                        wvt[:],
                        start=(k == 0), stop=(k == 15))
            for tb in range(6):
                dst = VA[tb][:].rearrange("q (h c) -> q h c", c=65)[:, :, 0:64]
                src = vps[tb][:].rearrange("q (h c) -> q h c", c=64)
                if tb % 2 == 0:
                    nc.vector.tensor_copy(dst, src)
                else:
                    nc.scalar.copy(dst, src)

        # ---------- Phase A3 + B (+C) ----------
        wop = top.enter_context(tc.tile_pool(name="wop", bufs=18))
        yst = top.enter_context(tc.tile_pool(name="yst", bufs=6))
        qtp = top.enter_context(tc.tile_pool(name="qtp", bufs=1))
        # QTP[m]: rows 0-63 = Q^T heads of kv-head 2m, rows 64-127 kv-head
        # 2m+1; cols = p*1024 + chunk*512 + hh*256 + q
        QTP = [qtp.tile([128, 2048], BF16, tag=f"qt{m}", name=f"qt{m}")
               for m in range(4)]
        otp = top.enter_context(tc.tile_pool(name="otp", bufs=1))
        OT = [[otp.tile([128, 256], BF16, tag=f"ot{p}_{i}", name=f"ot{p}_{i}")
               for i in range(16)] for p in range(P)]

        with ExitStack() as ph:
            xqp = ph.enter_context(tc.tile_pool(name="xqp", bufs=1))
            wst = ph.enter_context(tc.tile_pool(name="wst2", bufs=19))
            psA = ph.enter_context(tc.tile_pool(name="psA2", bufs=2, space="PSUM"))
            ep = ph.enter_context(tc.tile_pool(name="ep", bufs=4))
            lp = ph.enter_context(tc.tile_pool(name="lp", bufs=6))
            qkps = ph.enter_context(tc.tile_pool(name="qkps", bufs=2, space="PSUM"))
            pvps = ph.enter_context(tc.tile_pool(name="pvps", bufs=2, space="PSUM"))

            # wq DMAs for the prologue pair (slabs 0,1) go out first so the
            # sync queue has them ready before attention-free prologue runs
            def wq_dma(j, k):
                wt = wst.tile([128, 512], BF16, tag="wq")
                nc.sync.dma_start(
                    wt[:], wq[k * 128:(k + 1) * 128, j * 512:(j + 1) * 512])
                return wt

            wts0 = [wq_dma(0, k) for k in range(16)]

            xq = []
            for k in range(16):
                t = xqp.tile([128, 512], BF16, tag=f"xq{k}", name=f"xq{k}")
                nc.scalar.dma_start(t[:], xqT[k * 128:(k + 1) * 128, :])
                xq.append(t)

            pending = []
            state = {}

            def finish(st):
                pv, r2f, hp, p = st
                rsb = lp.tile([64, 512], F32, tag="rsb")
                nc.gpsimd.partition_broadcast(rsb[:], r2f[0:1, :])
                ot = OT[p][hp]
                for hh in range(2):
                    nc.vector.tensor_tensor(
                        ot[hh * 64:(hh + 1) * 64, :],
                        pv[0:64, hh * 256:(hh + 1) * 256],
                        rsb[0:64, hh * 256:(hh + 1) * 256],
                        mybir.AluOpType.mult)

            # --- A3: slab sl = wq cols [sl*256,(sl+1)*256) = heads
            # 4sl..4sl+3, all of kv-head n=sl -> QTP[sl//2] half sl%2.
            # Slabs are produced in pairs j = (2j, 2j+1) sharing wq tiles.
            def a3_mm1(k, fi, coff, qps, wt):
                nc.tensor.matmul(qps[fi][:],
                                 wt[:, coff + fi * 128:coff + (fi + 1) * 128],
                                 xq[k][:],
                                 start=(k == 0), stop=(k == 15))

            def a3_drain(sl, qps):
                mq, rh = sl // 2, sl % 2
                for fi in range(2):
                    for hh in range(2):
                        for p in range(P):
                            dst = QTP[mq][rh * 64:(rh + 1) * 64,
                                          p * 1024 + fi * 512 + hh * 256:
                                          p * 1024 + fi * 512 + (hh + 1) * 256]
                            src = qps[fi][hh * 64:(hh + 1) * 64,
                                          p * 256:(p + 1) * 256]
                            if hh != rh:
                                nc.vector.tensor_copy(dst, src)
                            else:
                                nc.scalar.copy(dst, src)

            def a3_pair_steps(j, wts=None):
                """Fill steps for slab pair (2j, 2j+1): 64 matmuls, 2 drains."""
                prefetched = wts is not None
                if not prefetched:
                    wts = [wq_dma(j, k) for k in range(3)]
                for sl_local in range(2):
                    sl = 2 * j + sl_local
                    coff = sl_local * 256
                    qps = [psA.tile([128, 512], F32, tag="pa",
                                    name=f"qps{sl}_{i}") for i in range(2)]
                    for k in range(16):
                        if not prefetched and sl_local == 0 and k + 3 < 16:
                            wts.append(wq_dma(j, k + 3))
                        for fi in range(2):
                            yield (a3_mm1, (k, fi, coff, qps, wts[k]))
                    yield ("drain", (sl, qps))

            def run_steps(steps, n):
                done = 0
                while done < n:
                    s = next(steps, None)
                    if s is None:
                        return
                    fn, args = s
                    if fn == "drain":
                        a3_drain(*args)
                    elif fn == "cmm":
                        c_mm(*args)
                        done += 1
                    elif fn == "cdma":
                        args[0]()
                    else:
                        fn(*args)
                        done += 1

            # --- C machinery (O-projection) ---
            pre_wo = []

            def c_mm(acc, p, mm, k, wt):
                # acc is an AP (PSUM region)
                nc.tensor.matmul(
                    acc,
                    OT[p][k][:, mm * 128:(mm + 1) * 128],
                    wt[:],
                    start=(k == 0), stop=(k == 15))

            # --- attention ---
            def attn_iter(m, p, chunk, steps):
                # retire the previous iteration's normalizations first so
                # their pv PSUM slots can be reused by this iteration
                if pending:
                    finish(pending.pop(0))
                if pending:
                    finish(pending.pop(0))
                pvA = pvps.tile([65, 512], F32, tag="pv")
                pvB = pvps.tile([65, 512], F32, tag="pv")
                qcols = slice(p * 1024 + chunk * 512, p * 1024 + (chunk + 1) * 512)
                es = []
                for tt in range(4):
                    run_steps(steps, 1)
                    toff = p * 256 + tt * 128
                    qk = qkps.tile([128, 1024], F32, tag="qk")
                    nc.tensor.matmul(qk[:, 0:512],
                                     KT[m][0:64, toff:toff + 128],
                                     QTP[m][0:64, qcols],
                                     start=True, stop=True)
                    nc.tensor.matmul(qk[:, 512:1024],
                                     KT[m][64:128, toff:toff + 128],
                                     QTP[m][64:128, qcols],
                                     start=True, stop=True)
                    e = ep.tile([128, 1024], BF16, tag="e")
                    nc.scalar.activation(e[:], qk[:],
                                         mybir.ActivationFunctionType.Exp,
                                         scale=float(SCALE))
                    es.append(e)
                    run_steps(steps, 1)
                    if tt >= 1:
                        tbp = 2 * p + tt - 1
                        nc.tensor.matmul(
                            pvA[:, 0:512],
                            VA[tbp][:, (2 * m) * 65:(2 * m) * 65 + 65],
                            es[tt - 1][:, 0:512],
                            start=(tt == 1), stop=False)
                        nc.tensor.matmul(
                            pvB[:, 0:512],
                            VA[tbp][:, (2 * m + 1) * 65:(2 * m + 1) * 65 + 65],
                            es[tt - 1][:, 512:1024],
                            start=(tt == 1), stop=False)
                    run_steps(steps, 2)
                tbp = 2 * p + 3
                nc.tensor.matmul(pvA[:, 0:512],
                                 VA[tbp][:, (2 * m) * 65:(2 * m) * 65 + 65],
                                 es[3][:, 0:512], start=False, stop=True)
                nc.tensor.matmul(pvB[:, 0:512],
                                 VA[tbp][:, (2 * m + 1) * 65:(2 * m + 1) * 65 + 65],
                                 es[3][:, 512:1024], start=False, stop=True)
                for pv, n in ((pvA, 2 * m), (pvB, 2 * m + 1)):
                    l2 = lp.tile([1, 512], F32, tag="l2")
                    nc.vector.tensor_copy(l2[:], pv[64:65, 0:512])
                    r2f = lp.tile([1, 512], F32, tag="r2f")
                    nc.vector.reciprocal_approx_fast(r2f[:], l2[:])
                    pending.append((pv, r2f, 2 * n + chunk, p))

            # --- B schedule ---
            # prologue: slab pair 0 (QTP[0]) with prefetched weights
            st = a3_pair_steps(0, wts0)
            run_steps(st, 10 ** 6)

            for m in range(4):
                if m < 3:
                    steps = a3_pair_steps(m + 1)
                else:
                    # final block: prefetch wo nn=0 and run C p=0 fills
                    def c_fill_steps():
                        for k in range(16):
                            wt = wop.tile([128, 512], BF16, tag="wo",
                                          name=f"wopre{k}")
                            pre_wo.append(wt)
                            yield ("cdma", ((lambda wt=wt, k=k:
                                             nc.sync.dma_start(
                                                 wt[:],
                                                 wo[k * 128:(k + 1) * 128, 0:512])),))
                        accs = [psA.tile([128, 512], F32, tag="pa",
                                         name=f"pc0_{i}")[:] for i in range(2)]
                        state["c0_accs"] = accs
                        for k in range(12):
                            for mm in range(2):
                                yield ("cmm", (accs[mm], 0, mm, k, pre_wo[k]))
                    steps = c_fill_steps()
                for p in range(P):
                    for chunk in range(2):
                        attn_iter(m, p, chunk, steps)
                run_steps(steps, 10 ** 6)
            while pending:
                finish(pending.pop(0))

            # ---------- Phase C: output projection ----------
            accs0 = state.pop("c0_accs")
            for k in range(12, 16):
                for mm in range(2):
                    c_mm(accs0[mm], 0, mm, k, pre_wo[k])
            # nn=0, p=1 (qk ring tiles span 2 banks; use halves as 2 accs)
            cq = qkps.tile([128, 1024], F32, tag="qk", name="pc0b")
            accs1 = [cq[:, 0:512], cq[:, 512:1024]]
            for k in range(16):
                for mm in range(2):
                    c_mm(accs1[mm], 1, mm, k, pre_wo[k])
            for p, accs in ((0, accs0), (1, accs1)):
                for mm in range(2):
                    yt = yst.tile([128, 512], BF16, tag="yt")
                    if mm == 0:
                        nc.vector.tensor_copy(yt[:], accs[mm])
                    else:
                        nc.scalar.copy(yt[:], accs[mm])
                    r0 = p * 256 + mm * 128
                    nc.sync.dma_start(y[r0:r0 + 128, 0:512], yt[:])
            for nn in range(1, 4):
                pa = [psA.tile([128, 512], F32, tag="pa",
                               name=f"pc{nn}_{i}")[:] for i in range(2)]
                cq = qkps.tile([128, 1024], F32, tag="qk", name=f"pc{nn}b")
                acc = [[pa[0], pa[1]], [cq[:, 0:512], cq[:, 512:1024]]]
                for k in range(16):
                    wt = wop.tile([128, 512], BF16, tag="wo")
                    nc.sync.dma_start(
                        wt[:], wo[k * 128:(k + 1) * 128, nn * 512:(nn + 1) * 512])
                    for p in range(P):
                        for mm in range(2):
                            c_mm(acc[p][mm], p, mm, k, wt)
                for p in range(P):
                    for mm in range(2):
                        yt = yst.tile([128, 512], BF16, tag="yt")
                        if mm == 0:
                            nc.vector.tensor_copy(yt[:], acc[p][mm])
                        else:
                            nc.scalar.copy(yt[:], acc[p][mm])
                        r0 = p * 256 + mm * 128
                        nc.sync.dma_start(
                            y[r0:r0 + 128, nn * 512:(nn + 1) * 512], yt[:])

    nc.compile()
    return nc


def _get_nc():
    if "nc" not in _CACHE:
        _CACHE["nc"] = _build()
    return _CACHE["nc"]


def _shard_inputs(x, wq, wkv, wo):
    import ml_dtypes

    bf16 = ml_dtypes.bfloat16
    x = np.asarray(x, dtype=np.float32)
    wq_b = np.ascontiguousarray(wq, dtype=np.float32).astype(bf16)
    wkv_b = np.ascontiguousarray(wkv, dtype=np.float32).astype(bf16)
    wo_b = np.ascontiguousarray(wo, dtype=np.float32).astype(bf16)

    in_maps = []
    for c in range(N_CORES):
        b = c // 4
        vA, vB = _VIEW_PAIRS[c % 4]
        xq_cols = np.concatenate(
            [np.ascontiguousarray(x[b, vA].T),
             np.ascontiguousarray(x[b, vB].T)], axis=1)
        kv_views = [(vA - 1) % V, (vA + 1) % V, (vB + 1) % V]
        xkv_cols = np.concatenate([x[b, u] for u in kv_views], axis=0).T
        in_maps.append({
            "xqT": xq_cols.astype(bf16),
            "xkvT": np.ascontiguousarray(xkv_cols).astype(bf16),
            "wq": wq_b, "wkv": wkv_b, "wo": wo_b,
        })
    return in_maps


def kernel(x, wq, wkv, wo):
    from concourse.bass_utils import run_bass_kernel_spmd

    nc = _get_nc()
    in_maps = _shard_inputs(x, wq, wkv, wo)
    res = run_bass_kernel_spmd(nc, in_maps, list(range(N_CORES)),
                               trace=False)
    out = np.empty((B, V, S, D), np.float32)
    for c in range(N_CORES):
        b = c // 4
        vA, vB = _VIEW_PAIRS[c % 4]
        yc = np.asarray(res.results[c]["y"], dtype=np.float32)
        out[b, vA] = yc[0:S]
        out[b, vB] = yc[S:2 * S]
    return out


# revision 17
# speedup vs baseline: 1.0431x; 1.0431x over previous
"""CrossViewAttention Trainium2 kernel (bf16), v4.

Sharding: B*V=16 instances over 8 cores, 2 per core paired as (v, v+2) so
both share view v+1's K/V projection -> per-core KV tokens 768 instead of
1024 (views v-1, v+1, v+3), cutting KV-projection matmul rows 25%.

QK^T contracts over only HD=64 (half the 128x128 PE idle), so the two
kv-heads of a block are packed into partition halves of KT/QTP and issued
as two matmuls in separate PE row-groups (tile_position row tiling) that
stream concurrently. Their two PSUM banks are one [128,1024] tile, so a
single scalar-engine exp covers both.

DMA: per-queue bandwidth is ~16 engines x min(row_bytes,2KB)/210ns, so
every stream is host-packed into >=2KB rows (k-slab pairs fused into one
row; x_kv into k-quads) and split across both hardware queues (sync +
scalar). A short warmup matmul chain runs during the initial DMA wait to
bring the PE out of its low P-state before real work arrives.

Pipeline per core:
  A1: KT[m][128,768] = wk.T @ x_kv^T   (kv-heads 2m / 2m+1 in halves)
  A2: VA[tb][128,8*65] = x_kv @ wv     (+ ones col for softmax denom)
  A3: QTP[m][128,2048] = wq.T @ x^T    (interleaved with attention)
  B:  per (m,p,chunk,tt): packed QK pair -> fused exp -> PV (lag-1)
  C:  y = O @ wo (first chunk interleaved into the last attention block)
"""
import numpy as np

B, V, S, D = 2, 8, 256, 2048
NH, NKV, KVR = 32, 8, 2
HD = D // NH  # 64
G = NH // NKV  # 4
N_CORES = 8
P = 2  # pairs per core
TKV = 768  # kv tokens per core (3 shared views)
SCALE = 1.0 / np.sqrt(HD)

# core -> (vA, vB) view pairs (vB = vA + 2, sharing view vA+1's KV)
_VIEW_PAIRS = [(0, 2), (1, 3), (4, 6), (5, 7)]

_CACHE = {}


def _pack_kpairs(a):
    """[2048, C] -> [1024, 2C]: row s*128+p = concat(a[2s*128+p], a[(2s+1)*128+p])."""
    c = a.shape[1]
    return np.ascontiguousarray(
        a.reshape(8, 2, 128, c).transpose(0, 2, 1, 3).reshape(1024, 2 * c))


def _build():
    import concourse.bass as bass
    import concourse.tile as tile
    import concourse.mybir as mybir
    from concourse import bacc
    from contextlib import ExitStack

    F32 = mybir.dt.float32
    BF16 = mybir.dt.bfloat16

    nc = bacc.Bacc("TRN2", target_bir_lowering=False, debug=False,
                   num_devices=N_CORES)
    # all DRAM layouts are host-packed so DMA rows are >= 2KB
    xqP = nc.dram_tensor("xqP", [1024, 1024], BF16, kind="ExternalInput").ap()
    xkvP = nc.dram_tensor("xkvP", [512, 3072], BF16, kind="ExternalInput").ap()
    wkP = nc.dram_tensor("wkP", [1024, 1024], BF16, kind="ExternalInput").ap()
    wvP = nc.dram_tensor("wvP", [1024, 1024], BF16, kind="ExternalInput").ap()
    wqP = nc.dram_tensor("wqP", [1024, 4096], BF16, kind="ExternalInput").ap()
    woP = nc.dram_tensor("woP", [1024, 4096], BF16, kind="ExternalInput").ap()
    y = nc.dram_tensor("y", [P * S, D], BF16, kind="ExternalOutput").ap()

    with tile.TileContext(nc) as tc, ExitStack() as top:
        ktp = top.enter_context(tc.tile_pool(name="ktp", bufs=1))
        vp = top.enter_context(tc.tile_pool(name="vp", bufs=1))

        # KT[m]: rows 0-63 kv-head 2m, rows 64-127 kv-head 2m+1; cols = t
        KT = [ktp.tile([128, TKV], BF16, tag=f"kt{m}", name=f"kt{m}")
              for m in range(4)]
        # VA[tb]: t-block tb (128 tokens) x (8 kv-heads x (64 hd + 1 one))
        VA = [vp.tile([128, 8 * 65], BF16, tag=f"va{tb}", name=f"va{tb}")
              for tb in range(6)]
        for tb in range(6):
            od = VA[tb][:].rearrange("q (h c) -> q h c", c=65)[:, :, 64:65]
            nc.gpsimd.memset(od, 1.0)

        # ---------- Phase A1/A2: K^T, V ----------
        with ExitStack() as ph:
            xkp = ph.enter_context(tc.tile_pool(name="xkp", bufs=1))
            wkp = ph.enter_context(tc.tile_pool(name="wkp", bufs=1))
            psA = ph.enter_context(tc.tile_pool(name="psA", bufs=8, space="PSUM"))

            # PE warmup: keep the array streaming during the initial DMA
            # wait so the clock ramps out of the low P-state.
            warm = xkp.tile([128, 512], BF16, tag="warm", name="warm")
            nc.vector.memset(warm[:], 1.0)
            wps = psA.tile([128, 512], F32, tag="pa", name="warmps")
            for _ in range(10):
                nc.tensor.matmul(wps[0:1, :], warm[:, 0:1], warm[:],
                                 start=True, stop=True)

            # x_kv: k 0-3 as singles (fast first arrival), then k-quads
            xq0 = [xkp.tile([128, TKV], BF16, tag=f"xkv0_{j}", name=f"xkv0_{j}")
                   for j in range(4)]
            xkq = [xkp.tile([128, 3072], BF16, tag=f"xkvq{q}", name=f"xkvq{q}")
                   for q in range(1, 4)]
            for j in range(4):
                nc.sync.dma_start(xq0[j][:],
                                  xkvP[0:128, j * TKV:(j + 1) * TKV])
            for q in range(1, 4):
                nc.sync.dma_start(xkq[q - 1][:], xkvP[q * 128:(q + 1) * 128, :])

            def xkv_ap(k):
                if k < 4:
                    return xq0[k][:]
                return xkq[k // 4 - 1][:].rearrange(
                    "p (j t) -> p j t", t=TKV)[:, k % 4, :]

            # wk / wv: k-pair packed tiles, resident
            wkt = [wkp.tile([128, 1024], BF16, tag=f"wk{s}", name=f"wk{s}")
                   for s in range(8)]
            wvt = [wkp.tile([128, 1024], BF16, tag=f"wv{s}", name=f"wv{s}")
                   for s in range(8)]
            for s in range(8):
                nc.scalar.dma_start(wkt[s][:], wkP[s * 128:(s + 1) * 128, :])
            for s in range(8):
                nc.scalar.dma_start(wvt[s][:], wvP[s * 128:(s + 1) * 128, :])

            # A1: KT accumulation, 8 psum banks (4 m x 2 halves of 384)
            kps = [psA.tile([128, 512], F32, tag="pa", name=f"kps{i}")
                   for i in range(8)]
            for k in range(16):
                xa = xkv_ap(k)
                wa = wkt[k // 2][:, (k % 2) * 512:(k % 2 + 1) * 512]
                for m in range(4):
                    for h in range(2):
                        nc.tensor.matmul(
                            kps[m * 2 + h][:, 0:384],
                            wa[:, m * 128:(m + 1) * 128],
                            xa[:, h * 384:(h + 1) * 384],
                            start=(k == 0), stop=(k == 15))
            for m in range(4):
                for h in range(2):
                    dst = KT[m][:, h * 384:(h + 1) * 384]
                    src = kps[m * 2 + h][:, 0:384]
                    if h == 0:
                        nc.vector.tensor_copy(dst, src)
                    else:
                        nc.scalar.copy(dst, src)

            # A2: V natural layout, 6 psum banks (t-blocks)
            vps = [psA.tile([128, 512], F32, tag="pa", name=f"pvv{tb}")
                   for tb in range(6)]
            for k in range(16):
                xa = xkv_ap(k)
                wa = wvt[k // 2][:, (k % 2) * 512:(k % 2 + 1) * 512]
                for tb in range(6):
                    nc.tensor.matmul(
                        vps[tb][:],
                        xa[:, tb * 128:(tb + 1) * 128],
                        wa,
                        start=(k == 0), stop=(k == 15))
            for tb in range(6):
                dst = VA[tb][:].rearrange("q (h c) -> q h c", c=65)[:, :, 0:64]
                src = vps[tb][:].rearrange("q (h c) -> q h c", c=64)
                if tb % 2 == 0:
                    nc.vector.tensor_copy(dst, src)
                else:
                    nc.scalar.copy(dst, src)

        # ---------- Phase A3 + B (+C) ----------
        wop = top.enter_context(tc.tile_pool(name="wop", bufs=10))
        yst = top.enter_context(tc.tile_pool(name="yst", bufs=6))
        qtp = top.enter_context(tc.tile_pool(name="qtp", bufs=1))
        # QTP[m]: rows 0-63 = Q^T heads of kv-head 2m, rows 64-127 kv-head
        # 2m+1; cols = p*1024 + chunk*512 + hh*256 + q
        QTP = [qtp.tile([128, 2048], BF16, tag=f"qt{m}", name=f"qt{m}")
               for m in range(4)]
        otp = top.enter_context(tc.tile_pool(name="otp", bufs=1))
        OT = [[otp.tile([128, 256], BF16, tag=f"ot{p}_{i}", name=f"ot{p}_{i}")
               for i in range(16)] for p in range(P)]

        with ExitStack() as ph:
            xqp = ph.enter_context(tc.tile_pool(name="xqp", bufs=1))
            wst = ph.enter_context(tc.tile_pool(name="wst2", bufs=11))
            psA = ph.enter_context(tc.tile_pool(name="psA2", bufs=2, space="PSUM"))
            ep = ph.enter_context(tc.tile_pool(name="ep", bufs=4))
            lp = ph.enter_context(tc.tile_pool(name="lp", bufs=6))
            qkps = ph.enter_context(tc.tile_pool(name="qkps", bufs=2, space="PSUM"))
            pvps = ph.enter_context(tc.tile_pool(name="pvps", bufs=2, space="PSUM"))

            # wq tiles: [128,1024] covers k-slabs 2s,2s+1 of col-block j
            def wq_dma(j, s):
                wt = wst.tile([128, 1024], BF16, tag="wq")
                nc.sync.dma_start(
                    wt[:], wqP[s * 128:(s + 1) * 128,
                               j * 1024:(j + 1) * 1024])
                return wt

            # prologue block j=0: all 8 tiles up front on the sync queue
            wts0 = [wq_dma(0, s) for s in range(8)]

            xqt = [xqp.tile([128, 1024], BF16, tag=f"xq{s}", name=f"xq{s}")
                   for s in range(8)]
            for s in range(8):
                nc.scalar.dma_start(xqt[s][:], xqP[s * 128:(s + 1) * 128, :])

            pending = []
            state = {}

            def finish(st):
                pv, r2f, hp, p = st
                rsb = lp.tile([64, 512], F32, tag="rsb")
                nc.gpsimd.partition_broadcast(rsb[:], r2f[0:1, :])
                ot = OT[p][hp]
                for hh in range(2):
                    nc.vector.tensor_tensor(
                        ot[hh * 64:(hh + 1) * 64, :],
                        pv[0:64, hh * 256:(hh + 1) * 256],
                        rsb[0:64, hh * 256:(hh + 1) * 256],
                        mybir.AluOpType.mult)

            # --- A3: slab sl = wq cols [sl*256,(sl+1)*256) = heads
            # 4sl..4sl+3, all of kv-head n=sl -> QTP[sl//2] half sl%2.
            def a3_mm1(k, fi, sl_local, qps, wt):
                coff = (k % 2) * 512 + sl_local * 256
                nc.tensor.matmul(qps[fi][:],
                                 wt[:, coff + fi * 128:coff + (fi + 1) * 128],
                                 xqt[k // 2][:, (k % 2) * 512:(k % 2 + 1) * 512],
                                 start=(k == 0), stop=(k == 15))

            def a3_drain(sl, qps):
                mq, rh = sl // 2, sl % 2
                for fi in range(2):
                    for hh in range(2):
                        for p in range(P):
                            dst = QTP[mq][rh * 64:(rh + 1) * 64,
                                          p * 1024 + fi * 512 + hh * 256:
                                          p * 1024 + fi * 512 + (hh + 1) * 256]
                            src = qps[fi][hh * 64:(hh + 1) * 64,
                                          p * 256:(p + 1) * 256]
                            if hh != rh:
                                nc.vector.tensor_copy(dst, src)
                            else:
                                nc.scalar.copy(dst, src)

            def a3_pair_steps(j, wts=None):
                """Fill steps for slab pair (2j, 2j+1): 64 matmuls + drains."""
                prefetched = wts is not None
                if not prefetched:
                    wts = [wq_dma(j, s) for s in range(3)]
                for sl_local in range(2):
                    sl = 2 * j + sl_local
                    qps = [psA.tile([128, 512], F32, tag="pa",
                                    name=f"qps{sl}_{i}") for i in range(2)]
                    for k in range(16):
                        if (not prefetched and sl_local == 0 and k % 2 == 0
                                and k // 2 + 3 < 8):
                            wts.append(wq_dma(j, k // 2 + 3))
                        for fi in range(2):
                            yield (a3_mm1, (k, fi, sl_local, qps, wts[k // 2]))
                    yield ("drain", (sl, qps))

            def run_steps(steps, n):
                done = 0
                while done < n:
                    s = next(steps, None)
                    if s is None:
                        return
                    fn, args = s
                    if fn == "drain":
                        a3_drain(*args)
                        done += 2  # let other tensor work cover the drain
                    elif fn == "cmm":
                        c_mm(*args)
                        done += 1
                    elif fn == "cdma":
                        args[0]()
                    else:
                        fn(*args)
                        done += 1

            # --- C machinery (O-projection) ---
            pre_wo = []

            def c_mm(acc, p, mm, k, wt):
                # acc is an AP (PSUM region); wt covers k-slabs 2s,2s+1
                nc.tensor.matmul(
                    acc,
                    OT[p][k][:, mm * 128:(mm + 1) * 128],
                    wt[:, (k % 2) * 512:(k % 2 + 1) * 512],
                    start=(k == 0), stop=(k == 15))

            # --- attention ---
            def attn_iter(m, p, chunk, steps):
                # retire the previous iteration's normalizations first so
                # their pv PSUM slots can be reused by this iteration
                if pending:
                    finish(pending.pop(0))
                if pending:
                    finish(pending.pop(0))
                pvA = pvps.tile([65, 512], F32, tag="pv")
                pvB = pvps.tile([65, 512], F32, tag="pv")
                qcols = slice(p * 1024 + chunk * 512, p * 1024 + (chunk + 1) * 512)
                es = []
                for tt in range(4):
                    run_steps(steps, 1)
                    toff = p * 256 + tt * 128
                    qk = qkps.tile([128, 1024], F32, tag="qk")
                    nc.tensor.matmul(qk[:, 0:512],
                                     KT[m][0:64, toff:toff + 128],
                                     QTP[m][0:64, qcols],
                                     start=True, stop=True)
                    nc.tensor.matmul(qk[:, 512:1024],
                                     KT[m][64:128, toff:toff + 128],
                                     QTP[m][64:128, qcols],
                                     start=True, stop=True)
                    e = ep.tile([128, 1024], BF16, tag="e")
                    nc.scalar.activation(e[:], qk[:],
                                         mybir.ActivationFunctionType.Exp,
                                         scale=float(SCALE))
                    es.append(e)
                    run_steps(steps, 1)
                    if tt >= 1:
                        tbp = 2 * p + tt - 1
                        nc.tensor.matmul(
                            pvA[:, 0:512],
                            VA[tbp][:, (2 * m) * 65:(2 * m) * 65 + 65],
                            es[tt - 1][:, 0:512],
                            start=(tt == 1), stop=False)
                        nc.tensor.matmul(
                            pvB[:, 0:512],
                            VA[tbp][:, (2 * m + 1) * 65:(2 * m + 1) * 65 + 65],
                            es[tt - 1][:, 512:1024],
                            start=(tt == 1), stop=False)
                    run_steps(steps, 2)
                tbp = 2 * p + 3
                nc.tensor.matmul(pvA[:, 0:512],
                                 VA[tbp][:, (2 * m) * 65:(2 * m) * 65 + 65],
                                 es[3][:, 0:512], start=False, stop=True)
                nc.tensor.matmul(pvB[:, 0:512],
                                 VA[tbp][:, (2 * m + 1) * 65:(2 * m + 1) * 65 + 65],
                                 es[3][:, 512:1024], start=False, stop=True)
                for pv, n in ((pvA, 2 * m), (pvB, 2 * m + 1)):
                    l2 = lp.tile([1, 512], F32, tag="l2")
                    nc.vector.tensor_copy(l2[:], pv[64:65, 0:512])
                    r2f = lp.tile([1, 512], F32, tag="r2f")
                    nc.vector.reciprocal_approx_fast(r2f[:], l2[:])
                    pending.append((pv, r2f, 2 * n + chunk, p))

            # --- B schedule ---
            st = a3_pair_steps(0, wts0)
            run_steps(st, 10 ** 6)

            for m in range(4):
                if m < 3:
                    steps = a3_pair_steps(m + 1)
                else:
                    # final block: prefetch wo nn=0 and run C p=0 fills
                    def c_fill_steps():
                        for s in range(8):
                            wt = wop.tile([128, 1024], BF16, tag="wo",
                                          name=f"wopre{s}")
                            pre_wo.append(wt)
                            yield ("cdma", ((lambda wt=wt, s=s:
                                             nc.sync.dma_start(
                                                 wt[:],
                                                 woP[s * 128:(s + 1) * 128,
                                                     0:1024])),))
                        accs = [psA.tile([128, 512], F32, tag="pa",
                                         name=f"pc0_{i}")[:] for i in range(2)]
                        state["c0_accs"] = accs
                        for k in range(12):
                            for mm in range(2):
                                yield ("cmm", (accs[mm], 0, mm, k,
                                               pre_wo[k // 2]))
                    steps = c_fill_steps()
                for p in range(P):
                    for chunk in range(2):
                        attn_iter(m, p, chunk, steps)
                run_steps(steps, 10 ** 6)
            while pending:
                finish(pending.pop(0))

            # ---------- Phase C: output projection ----------
            accs0 = state.pop("c0_accs")
            for k in range(12, 16):
                for mm in range(2):
                    c_mm(accs0[mm], 0, mm, k, pre_wo[k // 2])
            # nn=0, p=1 (qk ring tiles span 2 banks; use halves as 2 accs)
            cq = qkps.tile([128, 1024], F32, tag="qk", name="pc0b")
            accs1 = [cq[:, 0:512], cq[:, 512:1024]]
            for k in range(16):
                for mm in range(2):
                    c_mm(accs1[mm], 1, mm, k, pre_wo[k // 2])
            for p, accs in ((0, accs0), (1, accs1)):
                for mm in range(2):
                    yt = yst.tile([128, 512], BF16, tag="yt")
                    if mm == 0:
                        nc.vector.tensor_copy(yt[:], accs[mm])
                    else:
                        nc.scalar.copy(yt[:], accs[mm])
                    r0 = p * 256 + mm * 128
                    nc.sync.dma_start(y[r0:r0 + 128, 0:512], yt[:])
            for nn in range(1, 4):
                wts = []
                for s in range(8):
                    wt = wop.tile([128, 1024], BF16, tag="wo")
                    nc.scalar.dma_start(
                        wt[:], woP[s * 128:(s + 1) * 128,
                                   nn * 1024:(nn + 1) * 1024])
                    wts.append(wt)
                pa = [psA.tile([128, 512], F32, tag="pa",
                               name=f"pc{nn}_{i}")[:] for i in range(2)]
                cq = qkps.tile([128, 1024], F32, tag="qk", name=f"pc{nn}b")
                acc = [[pa[0], pa[1]], [cq[:, 0:512], cq[:, 512:1024]]]
                for k in range(16):
                    for p in range(P):
                        for mm in range(2):
                            c_mm(acc[p][mm], p, mm, k, wts[k // 2])
                for p in range(P):
                    for mm in range(2):
                        yt = yst.tile([128, 512], BF16, tag="yt")
                        if mm == 0:
                            nc.vector.tensor_copy(yt[:], acc[p][mm])
                        else:
                            nc.scalar.copy(yt[:], acc[p][mm])
                        r0 = p * 256 + mm * 128
                        nc.sync.dma_start(
                            y[r0:r0 + 128, nn * 512:(nn + 1) * 512], yt[:])

    nc.compile()
    return nc


def _get_nc():
    if "nc" not in _CACHE:
        _CACHE["nc"] = _build()
    return _CACHE["nc"]


def _shard_inputs(x, wq, wkv, wo):
    import ml_dtypes

    bf16 = ml_dtypes.bfloat16
    x = np.asarray(x, dtype=np.float32)
    wq32 = np.ascontiguousarray(wq, dtype=np.float32)
    wkv32 = np.ascontiguousarray(wkv, dtype=np.float32)
    wo32 = np.ascontiguousarray(wo, dtype=np.float32)

    wqP = np.ascontiguousarray(
        wq32.reshape(8, 2, 128, 4, 512).transpose(0, 2, 3, 1, 4)
        .reshape(1024, 4096)).astype(bf16)
    woP = np.ascontiguousarray(
        wo32.reshape(8, 2, 128, 4, 512).transpose(0, 2, 3, 1, 4)
        .reshape(1024, 4096)).astype(bf16)
    wkP = _pack_kpairs(wkv32[:, 0:512]).astype(bf16)
    wvP = _pack_kpairs(wkv32[:, 512:1024]).astype(bf16)

    in_maps = []
    for c in range(N_CORES):
        b = c // 4
        vA, vB = _VIEW_PAIRS[c % 4]
        xqT = np.concatenate(
            [np.ascontiguousarray(x[b, vA].T),
             np.ascontiguousarray(x[b, vB].T)], axis=1)  # [2048, 512]
        xqPc = _pack_kpairs(xqT).astype(bf16)  # [1024, 1024]
        kv_views = [(vA - 1) % V, (vA + 1) % V, (vB + 1) % V]
        xkvT = np.concatenate([x[b, u] for u in kv_views], axis=0).T  # [2048, 768]
        xkvPc = np.ascontiguousarray(
            xkvT.reshape(4, 4, 128, TKV).transpose(0, 2, 1, 3)
            .reshape(512, 4 * TKV)).astype(bf16)
        in_maps.append({
            "xqP": xqPc,
            "xkvP": xkvPc,
            "wqP": wqP, "wkP": wkP, "wvP": wvP, "woP": woP,
        })
    return in_maps


def kernel(x, wq, wkv, wo):
    from concourse.bass_utils import run_bass_kernel_spmd

    nc = _get_nc()
    in_maps = _shard_inputs(x, wq, wkv, wo)
    res = run_bass_kernel_spmd(nc, in_maps, list(range(N_CORES)),
                               trace=False)
    out = np.empty((B, V, S, D), np.float32)
    for c in range(N_CORES):
        b = c // 4
        vA, vB = _VIEW_PAIRS[c % 4]
        yc = np.asarray(res.results[c]["y"], dtype=np.float32)
        out[b, vA] = yc[0:S]
        out[b, vB] = yc[S:2 * S]
    return out


# revision 18
# speedup vs baseline: 1.1925x; 1.1432x over previous
"""CrossViewAttention Trainium2 kernel (bf16), v4.

Sharding: B*V=16 instances over 8 cores, 2 per core paired as (v, v+2) so
both share view v+1's K/V projection -> per-core KV tokens 768 instead of
1024 (views v-1, v+1, v+3), cutting KV-projection matmul rows 25%.

QK^T contracts over only HD=64 (half the 128x128 PE idle), so the two
kv-heads of a block are packed into partition halves of KT/QTP and issued
as two matmuls in separate PE row-groups (tile_position row tiling) that
stream concurrently. Their two PSUM banks are one [128,1024] tile, so a
single scalar-engine exp covers both.

DMA: per-queue bandwidth is ~16 engines x min(row_bytes,2KB)/210ns, so
every stream is host-packed into >=2KB rows (k-slab pairs fused into one
row; x_kv into k-quads) and split across both hardware queues (sync +
scalar). A short warmup matmul chain runs during the initial DMA wait to
bring the PE out of its low P-state before real work arrives.

Pipeline per core:
  A1: KT[m][128,768] = wk.T @ x_kv^T   (kv-heads 2m / 2m+1 in halves)
  A2: VA[tb][128,8*65] = x_kv @ wv     (+ ones col for softmax denom)
  A3: QTP[m][128,2048] = wq.T @ x^T    (interleaved with attention)
  B:  per (m,p,chunk,tt): packed QK pair -> fused exp -> PV (lag-1)
  C:  y = O @ wo (first chunk interleaved into the last attention block)
"""
import numpy as np

B, V, S, D = 2, 8, 256, 2048
NH, NKV, KVR = 32, 8, 2
HD = D // NH  # 64
G = NH // NKV  # 4
N_CORES = 8
P = 2  # pairs per core
TKV = 768  # kv tokens per core (3 shared views)
SCALE = 1.0 / np.sqrt(HD)

# core -> (vA, vB) view pairs (vB = vA + 2, sharing view vA+1's KV)
_VIEW_PAIRS = [(0, 2), (1, 3), (4, 6), (5, 7)]

_CACHE = {}


def _pack_kpairs(a):
    """[2048, C] -> [1024, 2C]: row s*128+p = concat(a[2s*128+p], a[(2s+1)*128+p])."""
    c = a.shape[1]
    return np.ascontiguousarray(
        a.reshape(8, 2, 128, c).transpose(0, 2, 1, 3).reshape(1024, 2 * c))


def _build():
    import concourse.bass as bass
    import concourse.tile as tile
    import concourse.mybir as mybir
    from concourse import bacc
    from contextlib import ExitStack

    F32 = mybir.dt.float32
    BF16 = mybir.dt.bfloat16

    nc = bacc.Bacc("TRN2", target_bir_lowering=False, debug=False,
                   num_devices=N_CORES)
    # all DRAM layouts are host-packed so DMA rows are >= 2KB
    xqP = nc.dram_tensor("xqP", [1024, 1024], BF16, kind="ExternalInput").ap()
    xkvP = nc.dram_tensor("xkvP", [512, 3072], BF16, kind="ExternalInput").ap()
    wkP = nc.dram_tensor("wkP", [1024, 1024], BF16, kind="ExternalInput").ap()
    wvP = nc.dram_tensor("wvP", [1024, 1024], BF16, kind="ExternalInput").ap()
    wqP = nc.dram_tensor("wqP", [1024, 4096], BF16, kind="ExternalInput").ap()
    woP = nc.dram_tensor("woP", [1024, 4096], BF16, kind="ExternalInput").ap()
    y = nc.dram_tensor("y", [P * S, D], BF16, kind="ExternalOutput").ap()

    with tile.TileContext(nc) as tc, ExitStack() as top:
        ktp = top.enter_context(tc.tile_pool(name="ktp", bufs=1))
        vp = top.enter_context(tc.tile_pool(name="vp", bufs=1))

        # KT[m]: rows 0-63 kv-head 2m, rows 64-127 kv-head 2m+1; cols = t
        KT = [ktp.tile([128, TKV], BF16, tag=f"kt{m}", name=f"kt{m}")
              for m in range(4)]
        # VA[tb]: t-block tb (128 tokens) x (8 kv-heads x (64 hd + 1 one))
        VA = [vp.tile([128, 8 * 65], BF16, tag=f"va{tb}", name=f"va{tb}")
              for tb in range(6)]
        for tb in range(6):
            od = VA[tb][:].rearrange("q (h c) -> q h c", c=65)[:, :, 64:65]
            nc.gpsimd.memset(od, 1.0)

        # ---------- Phase A1/A2: K^T, V ----------
        with ExitStack() as ph:
            xkp = ph.enter_context(tc.tile_pool(name="xkp", bufs=1))
            wkp = ph.enter_context(tc.tile_pool(name="wkp", bufs=1))
            psA = ph.enter_context(tc.tile_pool(name="psA", bufs=8, space="PSUM"))

            # PE warmup: keep the array streaming during the initial DMA
            # wait so the clock ramps out of the low P-state.
            warm = xkp.tile([128, 512], BF16, tag="warm", name="warm")
            nc.vector.memset(warm[:], 1.0)
            wps = psA.tile([128, 512], F32, tag="pa", name="warmps")
            for _ in range(3):
                nc.tensor.matmul(wps[0:1, :], warm[:, 0:1], warm[:],
                                 start=True, stop=True)

            # x_kv: k 0-3 as singles (fast first arrival), then k-quads
            xq0 = [xkp.tile([128, TKV], BF16, tag=f"xkv0_{j}", name=f"xkv0_{j}")
                   for j in range(4)]
            xkq = [xkp.tile([128, 3072], BF16, tag=f"xkvq{q}", name=f"xkvq{q}")
                   for q in range(1, 4)]
            for j in range(4):
                nc.sync.dma_start(xq0[j][:],
                                  xkvP[0:128, j * TKV:(j + 1) * TKV])
            for q in range(1, 4):
                nc.sync.dma_start(xkq[q - 1][:], xkvP[q * 128:(q + 1) * 128, :])

            def xkv_ap(k):
                if k < 4:
                    return xq0[k][:]
                return xkq[k // 4 - 1][:].rearrange(
                    "p (j t) -> p j t", t=TKV)[:, k % 4, :]

            # wk / wv: k-pair packed tiles, resident
            wkt = [wkp.tile([128, 1024], BF16, tag=f"wk{s}", name=f"wk{s}")
                   for s in range(8)]
            wvt = [wkp.tile([128, 1024], BF16, tag=f"wv{s}", name=f"wv{s}")
                   for s in range(8)]
            for s in range(8):
                nc.scalar.dma_start(wkt[s][:], wkP[s * 128:(s + 1) * 128, :])
            for s in range(8):
                nc.scalar.dma_start(wvt[s][:], wvP[s * 128:(s + 1) * 128, :])

            # A1: KT accumulation, 8 psum banks (4 m x 2 halves of 384)
            kps = [psA.tile([128, 512], F32, tag="pa", name=f"kps{i}")
                   for i in range(8)]
            for k in range(16):
                xa = xkv_ap(k)
                wa = wkt[k // 2][:, (k % 2) * 512:(k % 2 + 1) * 512]
                for m in range(4):
                    for h in range(2):
                        nc.tensor.matmul(
                            kps[m * 2 + h][:, 0:384],
                            wa[:, m * 128:(m + 1) * 128],
                            xa[:, h * 384:(h + 1) * 384],
                            start=(k == 0), stop=(k == 15))
            for m in range(4):
                for h in range(2):
                    nc.vector.tensor_copy(KT[m][:, h * 384:(h + 1) * 384],
                                          kps[m * 2 + h][:, 0:384])

            # A2: V natural layout, 6 psum banks (t-blocks)
            vps = [psA.tile([128, 512], F32, tag="pa", name=f"pvv{tb}")
                   for tb in range(6)]
            for k in range(16):
                xa = xkv_ap(k)
                wa = wvt[k // 2][:, (k % 2) * 512:(k % 2 + 1) * 512]
                for tb in range(6):
                    nc.tensor.matmul(
                        vps[tb][:],
                        xa[:, tb * 128:(tb + 1) * 128],
                        wa,
                        start=(k == 0), stop=(k == 15))
            for tb in range(6):
                dst = VA[tb][:].rearrange("q (h c) -> q h c", c=65)[:, :, 0:64]
                src = vps[tb][:].rearrange("q (h c) -> q h c", c=64)
                if tb % 2 == 0:
                    nc.vector.tensor_copy(dst, src)
                else:
                    nc.scalar.copy(dst, src)

        # ---------- Phase A3 + B (+C) ----------
        wop = top.enter_context(tc.tile_pool(name="wop", bufs=10))
        yst = top.enter_context(tc.tile_pool(name="yst", bufs=6))
        qtp = top.enter_context(tc.tile_pool(name="qtp", bufs=1))
        # QTP[m]: rows 0-63 = Q^T heads of kv-head 2m, rows 64-127 kv-head
        # 2m+1; cols = p*1024 + chunk*512 + hh*256 + q
        QTP = [qtp.tile([128, 2048], BF16, tag=f"qt{m}", name=f"qt{m}")
               for m in range(4)]
        otp = top.enter_context(tc.tile_pool(name="otp", bufs=1))
        OT = [[otp.tile([128, 256], BF16, tag=f"ot{p}_{i}", name=f"ot{p}_{i}")
               for i in range(16)] for p in range(P)]

        with ExitStack() as ph:
            xqp = ph.enter_context(tc.tile_pool(name="xqp", bufs=1))
            wst = ph.enter_context(tc.tile_pool(name="wst2", bufs=11))
            psA = ph.enter_context(tc.tile_pool(name="psA2", bufs=2, space="PSUM"))
            ep = ph.enter_context(tc.tile_pool(name="ep", bufs=4))
            lp = ph.enter_context(tc.tile_pool(name="lp", bufs=6))
            qkps = ph.enter_context(tc.tile_pool(name="qkps", bufs=2, space="PSUM"))
            pvps = ph.enter_context(tc.tile_pool(name="pvps", bufs=2, space="PSUM"))

            # wq tiles: [128,1024] covers k-slabs 2s,2s+1 of col-block j
            def wq_dma(j, s):
                wt = wst.tile([128, 1024], BF16, tag="wq")
                nc.sync.dma_start(
                    wt[:], wqP[s * 128:(s + 1) * 128,
                               j * 1024:(j + 1) * 1024])
                return wt

            # prologue block j=0: all 8 tiles up front on the sync queue
            wts0 = [wq_dma(0, s) for s in range(8)]

            xqt = [xqp.tile([128, 1024], BF16, tag=f"xq{s}", name=f"xq{s}")
                   for s in range(8)]
            for s in range(8):
                nc.scalar.dma_start(xqt[s][:], xqP[s * 128:(s + 1) * 128, :])

            pending = []
            state = {}

            def finish(st):
                pv, r2f, hp, p = st
                rsb = lp.tile([64, 512], F32, tag="rsb")
                nc.gpsimd.partition_broadcast(rsb[:], r2f[0:1, :])
                ot = OT[p][hp]
                for hh in range(2):
                    nc.vector.tensor_tensor(
                        ot[hh * 64:(hh + 1) * 64, :],
                        pv[0:64, hh * 256:(hh + 1) * 256],
                        rsb[0:64, hh * 256:(hh + 1) * 256],
                        mybir.AluOpType.mult)

            # --- A3: slab sl = wq cols [sl*256,(sl+1)*256) = heads
            # 4sl..4sl+3, all of kv-head n=sl -> QTP[sl//2] half sl%2.
            def a3_mm1(k, fi, sl_local, qps, wt):
                coff = (k % 2) * 512 + sl_local * 256
                nc.tensor.matmul(qps[fi][:],
                                 wt[:, coff + fi * 128:coff + (fi + 1) * 128],
                                 xqt[k // 2][:, (k % 2) * 512:(k % 2 + 1) * 512],
                                 start=(k == 0), stop=(k == 15))

            def a3_drain(sl, qps):
                mq, rh = sl // 2, sl % 2
                for fi in range(2):
                    for hh in range(2):
                        for p in range(P):
                            dst = QTP[mq][rh * 64:(rh + 1) * 64,
                                          p * 1024 + fi * 512 + hh * 256:
                                          p * 1024 + fi * 512 + (hh + 1) * 256]
                            src = qps[fi][hh * 64:(hh + 1) * 64,
                                          p * 256:(p + 1) * 256]
                            if hh != rh:
                                nc.vector.tensor_copy(dst, src)
                            else:
                                nc.scalar.copy(dst, src)

            def a3_pair_steps(j, wts=None):
                """Fill steps for slab pair (2j, 2j+1): 64 matmuls + drains."""
                prefetched = wts is not None
                if not prefetched:
                    wts = [wq_dma(j, s) for s in range(3)]
                for sl_local in range(2):
                    sl = 2 * j + sl_local
                    qps = [psA.tile([128, 512], F32, tag="pa",
                                    name=f"qps{sl}_{i}") for i in range(2)]
                    for k in range(16):
                        if (not prefetched and sl_local == 0 and k % 2 == 0
                                and k // 2 + 3 < 8):
                            wts.append(wq_dma(j, k // 2 + 3))
                        for fi in range(2):
                            yield (a3_mm1, (k, fi, sl_local, qps, wts[k // 2]))
                    yield ("drain", (sl, qps))

            def run_steps(steps, n):
                done = 0
                while done < n:
                    s = next(steps, None)
                    if s is None:
                        return
                    fn, args = s
                    if fn == "drain":
                        a3_drain(*args)
                        done += 2  # let other tensor work cover the drain
                    elif fn == "cmm":
                        c_mm(*args)
                        done += 1
                    elif fn == "cdma":
                        args[0]()
                    else:
                        fn(*args)
                        done += 1

            # --- C machinery (O-projection) ---
            pre_wo = []

            def c_mm(acc, p, mm, k, wt):
                # acc is an AP (PSUM region); wt covers k-slabs 2s,2s+1
                nc.tensor.matmul(
                    acc,
                    OT[p][k][:, mm * 128:(mm + 1) * 128],
                    wt[:, (k % 2) * 512:(k % 2 + 1) * 512],
                    start=(k == 0), stop=(k == 15))

            # --- attention ---
            def attn_iter(m, p, chunk, steps):
                # retire the previous iteration's normalizations first so
                # their pv PSUM slots can be reused by this iteration
                if pending:
                    finish(pending.pop(0))
                if pending:
                    finish(pending.pop(0))
                pvA = pvps.tile([65, 512], F32, tag="pv")
                pvB = pvps.tile([65, 512], F32, tag="pv")
                qcols = slice(p * 1024 + chunk * 512, p * 1024 + (chunk + 1) * 512)
                es = []
                for tt in range(4):
                    run_steps(steps, 1)
                    toff = p * 256 + tt * 128
                    qk = qkps.tile([128, 1024], F32, tag="qk")
                    nc.tensor.matmul(qk[:, 0:512],
                                     KT[m][0:64, toff:toff + 128],
                                     QTP[m][0:64, qcols],
                                     start=True, stop=True)
                    nc.tensor.matmul(qk[:, 512:1024],
                                     KT[m][64:128, toff:toff + 128],
                                     QTP[m][64:128, qcols],
                                     start=True, stop=True)
                    e = ep.tile([128, 1024], BF16, tag="e")
                    nc.scalar.activation(e[:], qk[:],
                                         mybir.ActivationFunctionType.Exp,
                                         scale=float(SCALE))
                    es.append(e)
                    run_steps(steps, 2)
                    if tt >= 1:
                        tbp = 2 * p + tt - 1
                        nc.tensor.matmul(
                            pvA[:, 0:512],
                            VA[tbp][:, (2 * m) * 65:(2 * m) * 65 + 65],
                            es[tt - 1][:, 0:512],
                            start=(tt == 1), stop=False)
                        nc.tensor.matmul(
                            pvB[:, 0:512],
                            VA[tbp][:, (2 * m + 1) * 65:(2 * m + 1) * 65 + 65],
                            es[tt - 1][:, 512:1024],
                            start=(tt == 1), stop=False)
                    run_steps(steps, 3)
                tbp = 2 * p + 3
                nc.tensor.matmul(pvA[:, 0:512],
                                 VA[tbp][:, (2 * m) * 65:(2 * m) * 65 + 65],
                                 es[3][:, 0:512], start=False, stop=True)
                nc.tensor.matmul(pvB[:, 0:512],
                                 VA[tbp][:, (2 * m + 1) * 65:(2 * m + 1) * 65 + 65],
                                 es[3][:, 512:1024], start=False, stop=True)
                for pv, n in ((pvA, 2 * m), (pvB, 2 * m + 1)):
                    l2 = lp.tile([1, 512], F32, tag="l2")
                    nc.vector.tensor_copy(l2[:], pv[64:65, 0:512])
                    r2f = lp.tile([1, 512], F32, tag="r2f")
                    nc.vector.reciprocal_approx_fast(r2f[:], l2[:])
                    pending.append((pv, r2f, 2 * n + chunk, p))

            # --- B schedule ---
            st = a3_pair_steps(0, wts0)
            run_steps(st, 10 ** 6)

            for m in range(4):
                if m < 3:
                    steps = a3_pair_steps(m + 1)
                else:
                    # final block: prefetch wo nn=0 and run C p=0 fills
                    def c_fill_steps():
                        for s in range(8):
                            wt = wop.tile([128, 1024], BF16, tag="wo",
                                          name=f"wopre{s}")
                            pre_wo.append(wt)
                            yield ("cdma", ((lambda wt=wt, s=s:
                                             nc.sync.dma_start(
                                                 wt[:],
                                                 woP[s * 128:(s + 1) * 128,
                                                     0:1024])),))
                        accs = [psA.tile([128, 512], F32, tag="pa",
                                         name=f"pc0_{i}")[:] for i in range(2)]
                        state["c0_accs"] = accs
                        for k in range(12):
                            for mm in range(2):
                                yield ("cmm", (accs[mm], 0, mm, k,
                                               pre_wo[k // 2]))
                    steps = c_fill_steps()
                for p in range(P):
                    for chunk in range(2):
                        attn_iter(m, p, chunk, steps)
                run_steps(steps, 10 ** 6)
            while pending:
                finish(pending.pop(0))

            # ---------- Phase C: output projection ----------
            accs0 = state.pop("c0_accs")
            for k in range(12, 16):
                for mm in range(2):
                    c_mm(accs0[mm], 0, mm, k, pre_wo[k // 2])
            # nn=0, p=1 (qk ring tiles span 2 banks; use halves as 2 accs)
            cq = qkps.tile([128, 1024], F32, tag="qk", name="pc0b")
            accs1 = [cq[:, 0:512], cq[:, 512:1024]]
            for k in range(16):
                for mm in range(2):
                    c_mm(accs1[mm], 1, mm, k, pre_wo[k // 2])
            for p, accs in ((0, accs0), (1, accs1)):
                for mm in range(2):
                    yt = yst.tile([128, 512], BF16, tag="yt")
                    if mm == 0:
                        nc.vector.tensor_copy(yt[:], accs[mm])
                    else:
                        nc.scalar.copy(yt[:], accs[mm])
                    r0 = p * 256 + mm * 128
                    nc.sync.dma_start(y[r0:r0 + 128, 0:512], yt[:])
            for nn in range(1, 4):
                wts = []
                for s in range(8):
                    wt = wop.tile([128, 1024], BF16, tag="wo")
                    nc.scalar.dma_start(
                        wt[:], woP[s * 128:(s + 1) * 128,
                                   nn * 1024:(nn + 1) * 1024])
                    wts.append(wt)
                pa = [psA.tile([128, 512], F32, tag="pa",
                               name=f"pc{nn}_{i}")[:] for i in range(2)]
                cq = qkps.tile([128, 1024], F32, tag="qk", name=f"pc{nn}b")
                acc = [[pa[0], pa[1]], [cq[:, 0:512], cq[:, 512:1024]]]
                for k in range(16):
                    for p in range(P):
                        for mm in range(2):
                            c_mm(acc[p][mm], p, mm, k, wts[k // 2])
                for p in range(P):
                    for mm in range(2):
                        yt = yst.tile([128, 512], BF16, tag="yt")
                        if mm == 0:
                            nc.vector.tensor_copy(yt[:], acc[p][mm])
                        else:
                            nc.scalar.copy(yt[:], acc[p][mm])
                        r0 = p * 256 + mm * 128
                        nc.sync.dma_start(
                            y[r0:r0 + 128, nn * 512:(nn + 1) * 512], yt[:])

    nc.compile()
    return nc


def _get_nc():
    if "nc" not in _CACHE:
        _CACHE["nc"] = _build()
    return _CACHE["nc"]


def _shard_inputs(x, wq, wkv, wo):
    import ml_dtypes

    bf16 = ml_dtypes.bfloat16
    x = np.asarray(x, dtype=np.float32)
    wq32 = np.ascontiguousarray(wq, dtype=np.float32)
    wkv32 = np.ascontiguousarray(wkv, dtype=np.float32)
    wo32 = np.ascontiguousarray(wo, dtype=np.float32)

    wqP = np.ascontiguousarray(
        wq32.reshape(8, 2, 128, 4, 512).transpose(0, 2, 3, 1, 4)
        .reshape(1024, 4096)).astype(bf16)
    woP = np.ascontiguousarray(
        wo32.reshape(8, 2, 128, 4, 512).transpose(0, 2, 3, 1, 4)
        .reshape(1024, 4096)).astype(bf16)
    wkP = _pack_kpairs(wkv32[:, 0:512]).astype(bf16)
    wvP = _pack_kpairs(wkv32[:, 512:1024]).astype(bf16)

    in_maps = []
    for c in range(N_CORES):
        b = c // 4
        vA, vB = _VIEW_PAIRS[c % 4]
        xqT = np.concatenate(
            [np.ascontiguousarray(x[b, vA].T),
             np.ascontiguousarray(x[b, vB].T)], axis=1)  # [2048, 512]
        xqPc = _pack_kpairs(xqT).astype(bf16)  # [1024, 1024]
        kv_views = [(vA - 1) % V, (vA + 1) % V, (vB + 1) % V]
        xkvT = np.concatenate([x[b, u] for u in kv_views], axis=0).T  # [2048, 768]
        xkvPc = np.ascontiguousarray(
            xkvT.reshape(4, 4, 128, TKV).transpose(0, 2, 1, 3)
            .reshape(512, 4 * TKV)).astype(bf16)
        in_maps.append({
            "xqP": xqPc,
            "xkvP": xkvPc,
            "wqP": wqP, "wkP": wkP, "wvP": wvP, "woP": woP,
        })
    return in_maps


def kernel(x, wq, wkv, wo):
    from concourse.bass_utils import run_bass_kernel_spmd

    nc = _get_nc()
    in_maps = _shard_inputs(x, wq, wkv, wo)
    res = run_bass_kernel_spmd(nc, in_maps, list(range(N_CORES)),
                               trace=False)
    out = np.empty((B, V, S, D), np.float32)
    for c in range(N_CORES):
        b = c // 4
        vA, vB = _VIEW_PAIRS[c % 4]
        yc = np.asarray(res.results[c]["y"], dtype=np.float32)
        out[b, vA] = yc[0:S]
        out[b, vB] = yc[S:2 * S]
    return out
